# revision 1
# baseline (speedup 1.0000x reference)
"""Trainium2 Bass kernel for nn_CrossAttUnit (ragged cross-attention unit), v2.

Math (per 64-token segment, N=262144 tokens total, H=256, D=64):
    yk = y_seg @ k            [64, 64]
    yq = yhat_seg @ q         [64, 64]
    M  = (yk @ yq.T) / 8      [64, 64]
    attn = softmax(M, axis=1) + 1e-6      (row softmax)
    W  = attn / attn.sum(axis=0)          (column normalize)
Output: [4096, 64, 64] float32.

Sharding: data-parallel over segments; core i handles tokens
[i*32768, (i+1)*32768) (512 whole segments). k, q replicated.

v2 dataflow (per core):
  Host splits y/yhat into bf16 hi+lo pairs (y = hi + lo to ~2^-17 rel) and
  lays them out chunk-major [2, n_loc, 128] so each (chunk, row-range) slice
  is a contiguous [rows, 128] block. k/q are split hi/lo too, zero-padded to
  [128, 2, 128] stationaries.

  Per 2048-token superblock: 8 transposing DMA loads (xbar) put yT tiles
  [128, 2, 2048] bf16 directly in SBUF -- no PE transposes, no PSUM copies.
  Per 512-token group (8 segments):
    ykT = k^T y^T via 6 accumulating bf16 matmuls (hi*hi + lo*hi + hi*lo,
    2 H-chunks each) -> PSUM [64(+64 zero), 512] fp32; same for yqT.
    PSUM->SBUF copy split DVE/ACT.
    M_seg = ykT_seg^T @ yqT_seg (fp32 matmuls, K=128 zero-padded) -> [128,4,64]
    row softmax: DVE max-reduce, ACT exp(+rowsum accum, one act table ever),
    DVE reciprocal, GPSIMD scale+eps.
    column normalize: PE block-diag colsum broadcast, DVE reciprocal,
    GPSIMD multiply.
  One output DMA per superblock.
"""

import numpy as np

N_TOTAL = 262144
H = 256
D = 64
L = 64  # seg_len
NCORES = 8
N_LOC = N_TOTAL // NCORES  # 32768
GROUP_TOK = 512  # 8 segments
SB_GROUPS = 4
SB_TOK = GROUP_TOK * SB_GROUPS  # 2048
SCALE = 0.125  # 1/sqrt(D)
EPS = 1e-6

_CACHE = {}


def _build_program(n_loc):
    import concourse.bacc as bacc
    import concourse.tile as tile
    from concourse import mybir

    f32 = mybir.dt.float32
    bf16 = mybir.dt.bfloat16
    FT = mybir.ActivationFunctionType
    OP = mybir.AluOpType

    nc = bacc.Bacc("TRN2", target_bir_lowering=False)

    yhi_d = nc.dram_tensor("yhi", [2, n_loc, 128], bf16, kind="ExternalInput")
    ylo_d = nc.dram_tensor("ylo", [2, n_loc, 128], bf16, kind="ExternalInput")
    hhi_d = nc.dram_tensor("hhi", [2, n_loc, 128], bf16, kind="ExternalInput")
    hlo_d = nc.dram_tensor("hlo", [2, n_loc, 128], bf16, kind="ExternalInput")
    khi_d = nc.dram_tensor("khi", [128, 256], bf16, kind="ExternalInput")
    klo_d = nc.dram_tensor("klo", [128, 256], bf16, kind="ExternalInput")
    qhi_d = nc.dram_tensor("qhi", [128, 256], bf16, kind="ExternalInput")
    qlo_d = nc.dram_tensor("qlo", [128, 256], bf16, kind="ExternalInput")
    bd_d = nc.dram_tensor("bdiag", [128, 128], f32, kind="ExternalInput")
    w_d = nc.dram_tensor("w", [n_loc, L], f32, kind="ExternalOutput")

    n_sb = n_loc // SB_TOK

    with tile.TileContext(nc) as tc:
        with (
            tc.tile_pool(name="consts", bufs=1) as consts,
            tc.tile_pool(name="yT", bufs=3) as yTp,
            tc.tile_pool(name="kq", bufs=2) as kqp,
            tc.tile_pool(name="attn", bufs=3) as attnp,
            tc.tile_pool(name="soft", bufs=2) as softp,
            tc.tile_pool(name="wout", bufs=3) as woutp,
            tc.tile_pool(name="stats", bufs=4) as statp,
            tc.tile_pool(name="ps_p", bufs=2, space="PSUM") as ps_p,
            tc.tile_pool(name="ps_m", bufs=2, space="PSUM") as ps_m,
            tc.tile_pool(name="ps_c", bufs=2, space="PSUM") as ps_c,
        ):
            khi_sb = consts.tile([128, 2, 128], bf16)
            klo_sb = consts.tile([128, 2, 128], bf16)
            qhi_sb = consts.tile([128, 2, 128], bf16)
            qlo_sb = consts.tile([128, 2, 128], bf16)
            bd_sb = consts.tile([128, 128], f32)
            nc.sync.dma_start(out=khi_sb[:], in_=khi_d.rearrange("p (c m) -> p c m", c=2))
            nc.sync.dma_start(out=klo_sb[:], in_=klo_d.rearrange("p (c m) -> p c m", c=2))
            nc.sync.dma_start(out=qhi_sb[:], in_=qhi_d.rearrange("p (c m) -> p c m", c=2))
            nc.sync.dma_start(out=qlo_sb[:], in_=qlo_d.rearrange("p (c m) -> p c m", c=2))
            nc.sync.dma_start(out=bd_sb[:], in_=bd_d[:])

            # 3-stage software pipeline across groups. Per iteration gi the
            # PE stream is [proj(gi), seg(gi-1), colsum(gi-2)], so PE never
            # waits on the cross-engine softmax chain; DVE/ACT/POOL work on
            # older groups while PE streams projections.
            n_groups = n_loc // GROUP_TOK
            yT_store = {}
            w_store = {}
            state = {}

            def issue_loads(sb):
                r0 = sb * SB_TOK
                yT = {}
                for nm, dram_t in (
                    ("yhi", yhi_d),
                    ("ylo", ylo_d),
                    ("hhi", hhi_d),
                    ("hlo", hlo_d),
                ):
                    t = yTp.tile([128, 2, SB_TOK], bf16, tag=nm)
                    for c in range(2):
                        nc.sync.dma_start(
                            out=t[:, c, :],
                            in_=dram_t[c, r0 : r0 + SB_TOK, :],
                            transpose=True,
                        )
                    yT[nm] = t
                yT_store[sb] = t_map = yT
                return t_map

            def pe_proj(gi):
                sb, g = divmod(gi, SB_GROUPS)
                yT = yT_store[sb]
                t0 = g * GROUP_TOK
                t1 = t0 + GROUP_TOK
                ykq_ps = ps_p.tile([128, 2, GROUP_TOK], f32, tag="ykq_ps")
                yk_terms = [
                    (khi_sb, yT["yhi"]),
                    (khi_sb, yT["ylo"]),
                    (klo_sb, yT["yhi"]),
                ]
                yq_terms = [
                    (qhi_sb, yT["hhi"]),
                    (qhi_sb, yT["hlo"]),
                    (qlo_sb, yT["hhi"]),
                ]
                for slot, terms in ((0, yk_terms), (1, yq_terms)):
                    mms = [
                        (kst[:, c, :], ymv[:, c, t0:t1])
                        for c in range(2)
                        for kst, ymv in terms
                    ]
                    for i, (kst_ap, ymv_ap) in enumerate(mms):
                        nc.tensor.matmul(
                            ykq_ps[:, slot, :],
                            kst_ap,
                            ymv_ap,
                            start=(i == 0),
                            stop=(i == len(mms) - 1),
                        )
                state[gi] = {"ykq_ps": ykq_ps}

            def pe_seg(gi):
                st = state[gi]
                ykq_sb = st["ykq_sb"]
                # segment matmuls: M[l, m] = sum_d yk[l, d] * yq[m, d]
                # K=128 with zero upper halves -- exact, all row-base-0
                M_ps = ps_m.tile([128, 4, L], f32, tag="M")
                for s in range(8):
                    sl = slice(s * L, (s + 1) * L)
                    nc.tensor.matmul(
                        M_ps[(s % 2) * 64 : (s % 2) * 64 + 64, s // 2, :],
                        ykq_sb[:, 0, sl],
                        ykq_sb[:, 1, sl],
                        start=True,
                        stop=True,
                    )
                st["M_ps"] = M_ps

            def pe_colsum(gi):
                st = state[gi]
                CS_ps = ps_c.tile([128, 4, L], f32, tag="CS")
                nc.tensor.matmul(
                    CS_ps[:], bd_sb[:], st["A_sb"][:], start=True, stop=True
                )
                st["CS_ps"] = CS_ps

            def softmax_front(gi):
                st = state[gi]
                M_ps = st["M_ps"]
                maxv = statp.tile([128, 4], f32, tag="maxv")
                nbias = statp.tile([128, 4], f32, tag="nbias")
                rowsum = statp.tile([128, 4], f32, tag="rowsum")
                rr = statp.tile([128, 4], f32, tag="rr")
                nc.vector.tensor_reduce(
                    maxv[:], M_ps[:], axis=mybir.AxisListType.X, op=OP.max
                )
                nc.vector.tensor_scalar_mul(nbias[:], maxv[:], -SCALE)
                A_sb = attnp.tile([128, 4, L], f32, tag="A")
                for b in range(4):
                    nc.scalar.activation(
                        A_sb[:, b, :],
                        M_ps[:, b, :],
                        FT.Exp,
                        bias=nbias[:, b : b + 1],
                        scale=SCALE,
                        accum_out=rowsum[:, b : b + 1],
                    )
                nc.vector.reciprocal(rr[:], rowsum[:])
                # attn' = E * (1/rowsum) + EPS
                for b in range(4):
                    nc.gpsimd.tensor_scalar(
                        A_sb[:, b, :],
                        A_sb[:, b, :],
                        rr[:, b : b + 1],
                        EPS,
                        op0=OP.mult,
                        op1=OP.add,
                    )
                st["A_sb"] = A_sb

            def norm_back(gi):
                sb, g = divmod(gi, SB_GROUPS)
                st = state.pop(gi)
                W_super = w_store[sb]
                rc_sb = softp.tile([128, 4, L], f32, tag="rc")
                nc.vector.reciprocal(rc_sb[:], st["CS_ps"][:])
                nc.gpsimd.tensor_tensor(
                    W_super[:, g * 4 : g * 4 + 4, :],
                    st["A_sb"][:],
                    rc_sb[:],
                    op=OP.mult,
                )
                if g == SB_GROUPS - 1:
                    r0 = sb * SB_TOK
                    nc.gpsimd.dma_start(
                        out=w_d[r0 : r0 + SB_TOK, :].rearrange(
                            "(t p) m -> p t m", p=128
                        ),
                        in_=W_super[:],
                    )
                    del yT_store[sb], w_store[sb]

            def copies(gi):
                st = state[gi]
                ykq_ps = st["ykq_ps"]
                ykq_sb = kqp.tile([128, 2, GROUP_TOK], f32, tag="ykq_sb")
                nc.vector.tensor_copy(ykq_sb[:, 0, :], ykq_ps[:, 0, :])
                nc.scalar.copy(ykq_sb[:, 1, :], ykq_ps[:, 1, :])
                st["ykq_sb"] = ykq_sb

            # Per iteration gi the PE stream is [proj(gi), seg(gi-1),
            # colsum(gi-2)]; the DVE/ACT softmax ops for gi-1 and the PSUM
            # copies for gi are emitted after the PE block so each engine's
            # in-order queue never blocks the chain.
            issue_loads(0)
            for gi in range(n_groups + 2):
                sb, g = divmod(gi, SB_GROUPS)
                if gi < n_groups:
                    if g == 0:
                        w_store[sb] = woutp.tile(
                            [128, SB_GROUPS * 4, L], f32, tag="W", name="W_super"
                        )
                        if sb + 1 < n_sb:
                            issue_loads(sb + 1)
                    pe_proj(gi)
                if gi >= 1 and gi - 1 < n_groups:
                    pe_seg(gi - 1)
                if gi >= 2 and gi - 2 < n_groups:
                    pe_colsum(gi - 2)
                if gi >= 1 and gi - 1 < n_groups:
                    softmax_front(gi - 1)
                if gi >= 2 and gi - 2 < n_groups:
                    norm_back(gi - 2)
                if gi < n_groups:
                    copies(gi)

    nc.compile()
    return nc


def _split_hi_lo(x):
    import ml_dtypes

    bf = ml_dtypes.bfloat16
    hi = x.astype(bf)
    lo = (x - hi.astype(np.float32)).astype(bf)
    return hi, lo


def _chunk_major(x):
    """[n, 256] -> [2, n, 128] contiguous."""
    n = x.shape[0]
    return np.ascontiguousarray(x.reshape(n, 2, 128).transpose(1, 0, 2))


def _pad_proj_hl(m):
    """[H, D] fp32 -> (hi, lo) each [128, 256] bf16: [p, c*128+j] = m_x[c*128+p, j],
    zero for j >= 64."""
    import ml_dtypes

    bf = ml_dtypes.bfloat16
    hi, lo = _split_hi_lo(np.asarray(m, dtype=np.float32))
    outs = []
    for part in (hi, lo):
        o = np.zeros((128, 256), dtype=bf)
        o[:, 0:64] = part[0:128, :]
        o[:, 128:192] = part[128:256, :]
        outs.append(o)
    return outs


def _consts():
    bdiag = np.zeros((128, 128), dtype=np.float32)
    bdiag[:64, :64] = 1.0
    bdiag[64:, 64:] = 1.0
    return bdiag


def _get_program(n_loc):
    if n_loc not in _CACHE:
        _CACHE[n_loc] = _build_program(n_loc)
    return _CACHE[n_loc]


def _prepare(yhat_embedding, y_embedding, k, q):
    nc = _get_program(N_LOC)
    bdiag = _consts()
    y = np.asarray(y_embedding, dtype=np.float32)
    yh = np.asarray(yhat_embedding, dtype=np.float32)
    yhi, ylo = _split_hi_lo(y)
    hhi, hlo = _split_hi_lo(yh)
    khi, klo = _pad_proj_hl(k)
    qhi, qlo = _pad_proj_hl(q)
    in_maps = []
    for i in range(NCORES):
        sl = slice(i * N_LOC, (i + 1) * N_LOC)
        in_maps.append(
            {
                "yhi": _chunk_major(yhi[sl]),
                "ylo": _chunk_major(ylo[sl]),
                "hhi": _chunk_major(hhi[sl]),
                "hlo": _chunk_major(hlo[sl]),
                "khi": khi,
                "klo": klo,
                "qhi": qhi,
                "qlo": qlo,
                "bdiag": bdiag,
            }
        )
    return nc, in_maps


def _run(yhat_embedding, y_embedding, k, q, trace=False):
    from concourse.bass_utils import run_bass_kernel_spmd

    nc, in_maps = _prepare(yhat_embedding, y_embedding, k, q)
    res = run_bass_kernel_spmd(nc, in_maps, core_ids=list(range(NCORES)), trace=trace)
    w = np.concatenate([r["w"] for r in res.results], axis=0)
    out = w.reshape(N_TOTAL // L, L, L)
    return out, res


def kernel(**inputs):
    yhat_embedding = inputs["yhat_embedding"]
    y_embedding = inputs["y_embedding"]
    k = inputs["k"]
    q = inputs["q"]
    seg_len = int(inputs.get("seg_len", L))
    assert seg_len == L, f"kernel hardcodes seg_len={L}, got {seg_len}"
    out, _ = _run(yhat_embedding, y_embedding, k, q, trace=False)
    return out



# revision 2
# speedup vs baseline: 1.0126x; 1.0126x over previous
"""Trainium2 Bass kernel for nn_CrossAttUnit, v3 — instruction-count-minimized.

Math (per 64-token segment, N=262144 tokens, H=256, D=64):
    yk = y_seg @ (k/8)        [64, 64]   (1/sqrt(D) folded into k on host)
    yq = yhat_seg @ q         [64, 64]
    M  = yk @ yq.T            [64, 64]
    attn = softmax(M, axis=1) + 1e-6
    W  = attn / attn.sum(axis=0)
Output: [4096, 64, 64] f32.

v3 vs v2 (this environment charges ~0.4-1.9 us PER INSTRUCTION, engines run
in parallel; so minimize per-engine instruction counts):
  - fp32 projections: y reconstructed on-chip as f32 (hi+lo add), k/q exact
    f32 stationaries -> 4 proj matmuls per 512-token group instead of 12
    bf16 hi/lo cross-term matmuls. Also improves precision (M err ~0.003
    vs ~0.016).
  - colsum batched over 2 groups (N=512 moving) -> 0.5 matmuls/group.
  - softmax: negated max-reduce (no separate nbias op), eps fused into the
    rr multiply, final normalize batched per 2 groups.
  - output store via HWDGE contiguous p-major DRAM layout (128 descriptors
    of 4KB vs v2's gpsimd SWDGE 2048x256B descriptor storm); host unpermutes.
  - inputs unchanged: bf16 hi/lo chunk-major, 8 transposed DMAs/superblock.
"""

import numpy as np

N_TOTAL = 262144
H = 256
D = 64
L = 64
NCORES = 8
N_LOC = N_TOTAL // NCORES  # 32768
GROUP_TOK = 512
SB_GROUPS = 4
SB_TOK = GROUP_TOK * SB_GROUPS  # 2048
SCALE = 0.125
EPS = 1e-6

_CACHE = {}


def _build_program(n_loc):
    import concourse.bacc as bacc
    import concourse.tile as tile
    from concourse import mybir

    f32 = mybir.dt.float32
    bf16 = mybir.dt.bfloat16
    FT = mybir.ActivationFunctionType
    OP = mybir.AluOpType

    nc = bacc.Bacc("TRN2", target_bir_lowering=False)

    yhi_d = nc.dram_tensor("yhi", [2, n_loc, 128], bf16, kind="ExternalInput")
    ylo_d = nc.dram_tensor("ylo", [2, n_loc, 128], bf16, kind="ExternalInput")
    hhi_d = nc.dram_tensor("hhi", [2, n_loc, 128], bf16, kind="ExternalInput")
    hlo_d = nc.dram_tensor("hlo", [2, n_loc, 128], bf16, kind="ExternalInput")
    kx_d = nc.dram_tensor("kx", [128, 2, 128], f32, kind="ExternalInput")
    qx_d = nc.dram_tensor("qx", [128, 2, 128], f32, kind="ExternalInput")
    bd_d = nc.dram_tensor("bdiag", [128, 128], f32, kind="ExternalInput")
    n_sb = n_loc // SB_TOK
    w_d = nc.dram_tensor("w", [128, n_sb * 16 * L], f32, kind="ExternalOutput")

    n_groups = n_loc // GROUP_TOK
    n_pairs = n_groups // 2

    with tile.TileContext(nc) as tc:
        with (
            tc.tile_pool(name="consts", bufs=1) as consts,
            tc.tile_pool(name="ybf", bufs=2) as ybfp,
            tc.tile_pool(name="yf32", bufs=2) as yf32p,
            tc.tile_pool(name="kq", bufs=3) as kqp,
            tc.tile_pool(name="attn", bufs=3) as attnp,
            tc.tile_pool(name="rcs", bufs=2) as rcp,
            tc.tile_pool(name="wout", bufs=2) as woutp,
            tc.tile_pool(name="stats", bufs=4) as statp,
            tc.tile_pool(name="ps_p", bufs=2, space="PSUM") as ps_p,
            tc.tile_pool(name="ps_m", bufs=2, space="PSUM") as ps_m,
            tc.tile_pool(name="ps_c", bufs=2, space="PSUM") as ps_c,
        ):
            kx_sb = consts.tile([128, 2, 128], f32)
            qx_sb = consts.tile([128, 2, 128], f32)
            bd_sb = consts.tile([128, 128], f32)
            nc.sync.dma_start(out=kx_sb[:], in_=kx_d[:])
            nc.sync.dma_start(out=qx_sb[:], in_=qx_d[:])
            nc.sync.dma_start(out=bd_sb[:], in_=bd_d[:])

            ybf_store = {}
            yf32_store = {}
            w_store = {}
            state = {}
            pair_state = {}

            def issue_loads(sb):
                r0 = sb * SB_TOK
                tiles = {}
                for nm, dram_t in (
                    ("yhi", yhi_d),
                    ("ylo", ylo_d),
                    ("hhi", hhi_d),
                    ("hlo", hlo_d),
                ):
                    t = ybfp.tile([128, 2, SB_TOK], bf16, tag=nm, name=nm)
                    for c in range(2):
                        nc.sync.dma_start(
                            out=t[:, c, :],
                            in_=dram_t[c, r0 : r0 + SB_TOK, :],
                            transpose=True,
                        )
                    tiles[nm] = t
                ybf_store[sb] = tiles
                yf32_store[sb] = {
                    "y": yf32p.tile([128, 2, SB_TOK], f32, tag="yf", name="yf"),
                    "h": yf32p.tile([128, 2, SB_TOK], f32, tag="hf", name="hf"),
                }

            def adds(gi):
                sb, g = divmod(gi, SB_GROUPS)
                src = ybf_store[sb]
                dst = yf32_store[sb]
                t0, t1 = g * GROUP_TOK, (g + 1) * GROUP_TOK
                nc.vector.tensor_tensor(
                    dst["y"][:, :, t0:t1],
                    src["yhi"][:, :, t0:t1],
                    src["ylo"][:, :, t0:t1],
                    op=OP.add,
                )
                nc.gpsimd.tensor_tensor(
                    dst["h"][:, :, t0:t1],
                    src["hhi"][:, :, t0:t1],
                    src["hlo"][:, :, t0:t1],
                    op=OP.add,
                )

            def pe_proj(gi):
                sb, g = divmod(gi, SB_GROUPS)
                yf = yf32_store[sb]
                t0, t1 = g * GROUP_TOK, (g + 1) * GROUP_TOK
                ykq_ps = ps_p.tile([128, 2, GROUP_TOK], f32, tag="ykq")
                for slot, (kst, mv) in enumerate(
                    ((kx_sb, yf["y"]), (qx_sb, yf["h"]))
                ):
                    for c in range(2):
                        nc.tensor.matmul(
                            ykq_ps[:, slot, :],
                            kst[:, c, :],
                            mv[:, c, t0:t1],
                            start=(c == 0),
                            stop=(c == 1),
                        )
                state[gi] = {"ykq_ps": ykq_ps}

            def copies(gi):
                st = state[gi]
                ykq_sb = kqp.tile([128, 2, GROUP_TOK], f32, tag="ykq_sb")
                nc.vector.tensor_copy(ykq_sb[:, 0, :], st["ykq_ps"][:, 0, :])
                nc.scalar.copy(ykq_sb[:, 1, :], st["ykq_ps"][:, 1, :])
                st["ykq_sb"] = ykq_sb

            def pe_seg(gi):
                st = state[gi]
                ykq_sb = st["ykq_sb"]
                M_ps = ps_m.tile([128, 4, L], f32, tag="M")
                for s in range(8):
                    sl = slice(s * L, (s + 1) * L)
                    nc.tensor.matmul(
                        M_ps[(s % 2) * 64 : (s % 2) * 64 + 64, s // 2, :],
                        ykq_sb[:, 0, sl],
                        ykq_sb[:, 1, sl],
                        start=True,
                        stop=True,
                    )
                st["M_ps"] = M_ps

            def softmax_front(gi):
                st = state.pop(gi)
                M_ps = st["M_ps"]
                j, half = divmod(gi, 2)
                if half == 0:
                    Apair = attnp.tile([128, 8, L], f32, tag="A")
                    pair_state[j] = {"A": Apair}
                else:
                    Apair = pair_state[j]["A"]
                A = Apair[:, half * 4 : half * 4 + 4, :]
                nmax = statp.tile([128, 4], f32, tag="nmax")
                rowsum = statp.tile([128, 4], f32, tag="rowsum")
                rr = statp.tile([128, 4], f32, tag="rr")
                nc.vector.tensor_reduce(
                    nmax[:], M_ps[:], axis=mybir.AxisListType.X, op=OP.max, negate=True
                )
                for b in range(4):
                    nc.scalar.activation(
                        A[:, b, :],
                        M_ps[:, b, :],
                        FT.Exp,
                        bias=nmax[:, b : b + 1],
                        scale=1.0,
                        accum_out=rowsum[:, b : b + 1],
                    )
                nc.vector.reciprocal(rr[:], rowsum[:])
                # T' = E * rr + EPS (rr broadcast along m if supported)
                rrb = None
                try:
                    rrb = rr[:].unsqueeze(2).broadcast_to((128, 4, L))
                except Exception:
                    rrb = None
                if rrb is not None:
                    nc.gpsimd.tensor_tensor(A[:], A[:], rrb, op=OP.mult)
                    nc.gpsimd.tensor_scalar_add(A[:], A[:], EPS)
                else:
                    for b in range(4):
                        nc.gpsimd.tensor_scalar(
                            A[:, b, :],
                            A[:, b, :],
                            rr[:, b : b + 1],
                            EPS,
                            op0=OP.mult,
                            op1=OP.add,
                        )

            def pe_colsum(j):
                ps = pair_state[j]
                CS_ps = ps_c.tile([128, 8, L], f32, tag="CS")
                nc.tensor.matmul(CS_ps[:], bd_sb[:], ps["A"][:], start=True, stop=True)
                ps["CS_ps"] = CS_ps

            def norm_back(j):
                ps = pair_state.pop(j)
                sb, jj = divmod(j, 2)
                rc_sb = rcp.tile([128, 8, L], f32, tag="rc")
                nc.vector.reciprocal(rc_sb[:], ps["CS_ps"][:])
                W_super = w_store[sb]
                nc.vector.tensor_tensor(
                    W_super[:, jj * 8 : jj * 8 + 8, :],
                    ps["A"][:],
                    rc_sb[:],
                    op=OP.mult,
                )

            def store_w(sb):
                nc.sync.dma_start(
                    out=w_d[:, sb * 16 * L : (sb + 1) * 16 * L].rearrange(
                        "p (t m) -> p t m", t=16
                    ),
                    in_=w_store.pop(sb)[:],
                )

            issue_loads(0)
            adds(0)
            adds(1)
            for gi in range(n_groups + 3):
                sb, g = divmod(gi, SB_GROUPS)
                if gi < n_groups:
                    if g == 0:
                        w_store[sb] = woutp.tile([128, 16, L], f32, tag="W", name="W")
                        if sb + 1 < n_sb:
                            issue_loads(sb + 1)
                    pe_proj(gi)
                if gi >= 1 and gi - 1 < n_groups:
                    pe_seg(gi - 1)
                if gi >= 3 and (gi - 3) % 2 == 0 and (gi - 3) // 2 < n_pairs:
                    pe_colsum((gi - 3) // 2)
                if gi >= 1 and gi - 1 < n_groups:
                    softmax_front(gi - 1)
                if gi >= 4 and (gi - 4) % 2 == 0 and (gi - 4) // 2 < n_pairs:
                    j = (gi - 4) // 2
                    norm_back(j)
                    if j % 2 == 1:
                        store_w(j // 2)
                if gi < n_groups:
                    copies(gi)
                if gi + 2 < n_groups:
                    adds(gi + 2)

    nc.compile()
    return nc


def _split_hi_lo(x):
    import ml_dtypes

    bf = ml_dtypes.bfloat16
    hi = x.astype(bf)
    lo = (x - hi.astype(np.float32)).astype(bf)
    return hi, lo


def _chunk_major(x):
    """[n, 256] -> [2, n, 128] contiguous."""
    n = x.shape[0]
    return np.ascontiguousarray(x.reshape(n, 2, 128).transpose(1, 0, 2))


def _pad_proj_f32(m, scale=1.0):
    """[256, 64] f32 -> [128, 2, 128]: out[p, c, j] = m[c*128+p, j]*scale, 0 pad."""
    m = np.asarray(m, dtype=np.float32) * scale
    o = np.zeros((128, 2, 128), dtype=np.float32)
    o[:, 0, 0:64] = m[0:128, :]
    o[:, 1, 0:64] = m[128:256, :]
    return o


def _consts():
    bdiag = np.zeros((128, 128), dtype=np.float32)
    bdiag[:64, :64] = 1.0
    bdiag[64:, 64:] = 1.0
    return bdiag


def _get_program(n_loc):
    if n_loc not in _CACHE:
        _CACHE[n_loc] = _build_program(n_loc)
    return _CACHE[n_loc]


def _prepare(yhat_embedding, y_embedding, k, q):
    nc = _get_program(N_LOC)
    bdiag = _consts()
    y = np.asarray(y_embedding, dtype=np.float32)
    yh = np.asarray(yhat_embedding, dtype=np.float32)
    yhi, ylo = _split_hi_lo(y)
    hhi, hlo = _split_hi_lo(yh)
    kx = _pad_proj_f32(k, scale=SCALE)
    qx = _pad_proj_f32(q)
    in_maps = []
    for i in range(NCORES):
        sl = slice(i * N_LOC, (i + 1) * N_LOC)
        in_maps.append(
            {
                "yhi": _chunk_major(yhi[sl]),
                "ylo": _chunk_major(ylo[sl]),
                "hhi": _chunk_major(hhi[sl]),
                "hlo": _chunk_major(hlo[sl]),
                "kx": kx,
                "qx": qx,
                "bdiag": bdiag,
            }
        )
    return nc, in_maps


def _unpermute(w):
    """[128, n_sb*16*64] -> [n_loc, 64]."""
    n_sb = N_LOC // SB_TOK
    return np.ascontiguousarray(
        w.reshape(128, n_sb, 16, L).transpose(1, 2, 0, 3)
    ).reshape(N_LOC, L)


def _run(yhat_embedding, y_embedding, k, q, trace=False):
    from concourse.bass_utils import run_bass_kernel_spmd

    nc, in_maps = _prepare(yhat_embedding, y_embedding, k, q)
    res = run_bass_kernel_spmd(nc, in_maps, core_ids=list(range(NCORES)), trace=trace)
    w = np.concatenate([_unpermute(r["w"]) for r in res.results], axis=0)
    out = w.reshape(N_TOTAL // L, L, L)
    return out, res


def kernel(**inputs):
    yhat_embedding = inputs["yhat_embedding"]
    y_embedding = inputs["y_embedding"]
    k = inputs["k"]
    q = inputs["q"]
    seg_len = int(inputs.get("seg_len", L))
    assert seg_len == L, f"kernel hardcodes seg_len={L}, got {seg_len}"
    out, _ = _run(yhat_embedding, y_embedding, k, q, trace=False)
    return out
